# revision 14
# baseline (speedup 1.0000x reference)
"""Trainium2 Bass kernel for AttentionLayerWithPrev.

Reference computation (B=4, L=S=1024, D=1024, H=16, DK=DV=64):
    Q = queries @ Wq + bq;  K = keys @ Wk + bk;  V = values @ Wv + bv
    scores = einsum(Q, K) / sqrt(DK) + prev_logits          [B, H, L, S]
    attn   = softmax(where(mask, -1e9, scores), axis=-1)    [B, H, L, S]
    out    = einsum(attn, V) @ Wo + bo                      [B, L, D]
Returns (out, attn, scores).

Sharding: 8 cores; core c handles batch b = c//2 and head group g = c%2
(heads 8g..8g+7).  Each core computes its slice of scores/attn fully and a
partial output projection (its 8 heads' contribution); the host sums the two
partials per batch and adds bo.
"""

import numpy as np

import concourse.bass as bass
import concourse.bacc as bacc
import concourse.tile as tile
from concourse import mybir
from concourse.bass import ts
from concourse.masks import make_identity

P = 128

# Full-problem constants (hardcoded per harness contract).
B, L, S = 4, 1024, 1024
D, H, DK, DV = 1024, 16, 64, 64
N_CORES = 8
HPC = H // (N_CORES // B)  # heads per core = 8

F32 = mybir.dt.float32


def build_nc(
    L=L,
    S=S,
    D=D,
    HPC=HPC,
    DK=DK,
    DV=DV,
    mm_dt=F32,
    enable_asserts=False,
):
    """Build the per-core Bass program.

    Per-core external tensors (heads pre-sliced / weights pre-scaled on host):
      q, k, v      [L|S, D]        inputs for this core's batch
      prev         [HPC, L, S]     prev_logits slice (this core's heads)
      wq, wk       [D, HPC*DK]     pre-scaled by 1/sqrt(DK) (wq only)
      wv           [D, HPC*DV]
      wo           [HPC*DV, D]
      bq, bk       [HPC*DK]        (bq pre-scaled by 1/sqrt(DK))
      bv           [HPC*DV]
      scores_o     [HPC, L, S]     output
      attn_o       [HPC, L, S]     output
      out_o        [L, D]          partial output projection (no bo)
    """
    HDK = HPC * DK
    HDV = HPC * DV
    assert D % P == 0 and L % P == 0 and S % P == 0
    assert HDK % P == 0 and HDV % P == 0 and DK == 64 and DV == 64
    DC = D // P      # contraction chunks for projections
    LT = L // P      # lq tiles
    SC = S // P      # s chunks (transpose/AV)
    MQ = HDK // P    # dk tiles of Qt/Kt (2 heads per tile)
    KO = HDV // P    # contraction chunks for out projection
    NS = min(512, S)   # scores matmul free chunk
    NL = min(512, L)   # projection free chunk
    ND = min(512, D)   # out projection free chunk
    GT = min(4, LT)  # transpose batch (input transposes)
    GS = min(4, SC)  # transpose batch (attn transposes)

    nc = bacc.Bacc(
        "TRN2",
        target_bir_lowering=False,
        debug=False,
        enable_asserts=enable_asserts,
        num_devices=1,
    )

    def mk_in(name, shape):
        return nc.dram_tensor(name, shape, F32, kind="ExternalInput").ap()

    def mk_out(name, shape):
        return nc.dram_tensor(name, shape, F32, kind="ExternalOutput").ap()

    q = mk_in("q", [L, D])
    k = mk_in("k", [S, D])
    v = mk_in("v", [S, D])
    prev = mk_in("prev", [HPC, L, S])
    wq = mk_in("wq", [D, HDK])
    wk = mk_in("wk", [D, HDK])
    wv = mk_in("wv", [D, HDV])
    wo = mk_in("wo", [HDV, D])
    bq = mk_in("bq", [HDK])
    bk = mk_in("bk", [HDK])
    bv = mk_in("bv", [HDV])
    scores_o = mk_out("scores_o", [HPC, L, S])
    attn_o = mk_out("attn_o", [HPC, L, S])
    out_o = mk_out("out_o", [L, D])

    Exp = mybir.ActivationFunctionType.Exp

    def cast(ap):
        return ap.bitcast(mm_dt) if mm_dt != F32 else ap

    with tile.TileContext(nc) as tc:
        with (
            tc.tile_pool(name="consts", bufs=1) as consts,
            tc.tile_pool(name="persist", bufs=1) as persist,
            tc.tile_pool(name="prevp", bufs=4) as prevp,
        ):
            ident = consts.tile([P, P], F32)
            make_identity(nc, ident)

            # Persistent SBUF tensors for the attention phase.
            qt = persist.tile([P, MQ, L], F32)     # Q^T  [dk, l]
            kt = persist.tile([P, MQ, S], F32)     # K^T  [dk, s]
            vn = persist.tile([P, SC, HDV], F32)   # V    [s, hdv]
            bq_sb = persist.tile([P, MQ], F32)
            bk_sb = persist.tile([P, MQ], F32)
            bv_bc = persist.tile([P, HDV], F32)    # bv broadcast over partitions

            nc.gpsimd.dma_start(
                out=bq_sb, in_=bq.rearrange("(m p) -> p m", p=P)
            )
            nc.gpsimd.dma_start(
                out=bk_sb, in_=bk.rearrange("(m p) -> p m", p=P)
            )
            bv_bcast_ap = bass.AP(
                tensor=bv.tensor, offset=bv.offset, ap=[[0, P], *bv.ap]
            )
            nc.gpsimd.dma_start(out=bv_bc, in_=bv_bcast_ap)

            # ---------------- Stage 1: transposes + QKV projections --------
            with (
                tc.tile_pool(name="wpool", bufs=2) as wpool,
                tc.tile_pool(name="xpool", bufs=2) as xpool,
                tc.tile_pool(name="xtpool", bufs=1) as xtpool,
                tc.tile_pool(name="tpsum", bufs=2, space="PSUM") as tpsum,
                tc.tile_pool(name="ppsum", bufs=2, space="PSUM") as ppsum,
            ):
                for xi, (src, wsrc) in enumerate([(q, wq), (k, wk), (v, wv)]):
                    xlt = (L if xi == 0 else S) // P
                    w_sb = wpool.tile([P, DC, HDK], F32, tag="w")
                    nc.sync.dma_start(
                        out=w_sb, in_=wsrc.rearrange("(c p) n -> p c n", p=P)
                    )
                    x_sb = xpool.tile([P, xlt, D], F32, tag="x")
                    nc.sync.dma_start(
                        out=x_sb, in_=src.rearrange("(i p) d -> p i d", p=P)
                    )
                    # Transpose x -> xT [d, l]
                    xT = xtpool.tile([P, DC, xlt * P], F32, tag="xt")
                    for j in range(DC):
                        for g in range(0, xlt, GT):
                            gn = min(GT, xlt - g)
                            pt = tpsum.tile([P, GT * P], F32, tag="pt")
                            for ii in range(gn):
                                nc.tensor.transpose(
                                    pt[:, ts(ii, P)],
                                    x_sb[:, g + ii, ts(j, P)],
                                    ident,
                                )
                            nc.scalar.copy(
                                xT[:, j, g * P : (g + gn) * P],
                                pt[:, : gn * P],
                            )
                    if xi < 2:
                        # Q^T / K^T = Wq^T @ x^T: out [dk, l]
                        dst = qt if xi == 0 else kt
                        bias = bq_sb if xi == 0 else bk_sb
                        nchunks = (xlt * P) // NL
                        for m in range(MQ):
                            for g in range(nchunks):
                                ps = ppsum.tile([P, NL], F32, tag="ps")
                                for kk in range(DC):
                                    nc.tensor.matmul(
                                        ps,
                                        lhsT=cast(w_sb[:, kk, ts(m, P)]),
                                        rhs=cast(xT[:, kk, ts(g, NL)]),
                                        start=(kk == 0),
                                        stop=(kk == DC - 1),
                                    )
                                nc.vector.tensor_scalar_add(
                                    dst[:, m, ts(g, NL)],
                                    in0=ps,
                                    scalar1=bias[:, m : m + 1],
                                )
                    else:
                        # V = v @ Wv: out [s, hdv]
                        for m in range(SC):
                            ps = ppsum.tile([P, HDV], F32, tag="psv")
                            for kk in range(DC):
                                nc.tensor.matmul(
                                    ps,
                                    lhsT=cast(xT[:, kk, ts(m, P)]),
                                    rhs=cast(w_sb[:, kk, :HDV]),
                                    start=(kk == 0),
                                    stop=(kk == DC - 1),
                                )
                            nc.vector.tensor_add(vn[:, m, :], ps, bv_bc)

            # ---------------- Stage 2: attention -------------------------
            with (
                tc.tile_pool(name="wop", bufs=1) as wop,
                tc.tile_pool(name="scorep", bufs=3) as scorep,
                tc.tile_pool(name="expp", bufs=2) as expp,
                tc.tile_pool(name="attnp", bufs=3) as attnp,
                tc.tile_pool(name="attntp", bufs=3) as attntp,
                tc.tile_pool(name="avtp", bufs=2) as avtp,
                tc.tile_pool(name="outp", bufs=2) as outp,
                tc.tile_pool(name="smol", bufs=12) as smol,
                tc.tile_pool(name="spsum", bufs=2, space="PSUM") as spsum,
                tc.tile_pool(name="tpsum2", bufs=2, space="PSUM") as tpsum2,
                tc.tile_pool(name="avpsum", bufs=1, space="PSUM") as avpsum,
                tc.tile_pool(name="opsum", bufs=1, space="PSUM") as opsum,
            ):
                wo_sb = wop.tile([P, KO, D], F32)  # Wo [hdv, d]
                nc.sync.dma_start(
                    out=wo_sb, in_=wo.rearrange("(c p) n -> p c n", p=P)
                )
                for t in range(LT):
                    avt = avtp.tile([P, KO, P], F32, tag="avt")
                    for hp in range(HPC // 2):
                        ps_av = avpsum.tile([P, P], F32, tag="av")
                        # scores matmuls for the head pair, emitted adjacently
                        # so the K=64 matmuls pack into different PE row
                        # groups (partition offsets 0/64) and run ~2x.
                        ps_pair = [
                            spsum.tile([P, S], F32, tag="s", name=f"ps_s{i}")
                            for i in range(2)
                        ]
                        for g in range(S // NS):
                            for h2 in range(2):
                                po = 64 * h2
                                nc.tensor.matmul(
                                    ps_pair[h2][:, ts(g, NS)],
                                    lhsT=cast(qt[po : po + 64, hp, ts(t, P)]),
                                    rhs=cast(kt[po : po + 64, hp, ts(g, NS)]),
                                    start=True,
                                    stop=True,
                                    tile_position=(po, 0),
                                )
                        attnts = [None, None]
                        for h2 in range(2):
                            h = 2 * hp + h2
                            po = 64 * h2
                            ps_s = ps_pair[h2]
                            prev_sb = prevp.tile([P, S], F32, tag="prev")
                            nc.sync.dma_start(
                                out=prev_sb, in_=prev[h, ts(t, P), :]
                            )
                            scores_sb = scorep.tile([P, S], F32, tag="sc")
                            nc.vector.tensor_add(scores_sb, ps_s, prev_sb)
                            nc.sync.dma_start(
                                out=scores_o[h, ts(t, P), :], in_=scores_sb
                            )
                            # softmax over free dim
                            mx = smol.tile([P, 1], F32, tag="mx")
                            nc.vector.reduce_max(
                                mx, scores_sb, axis=mybir.AxisListType.X
                            )
                            negmx = smol.tile([P, 1], F32, tag="negmx")
                            nc.vector.tensor_scalar_mul(negmx, in0=mx, scalar1=-1.0)
                            sumv = smol.tile([P, 1], F32, tag="sumv")
                            exp_sb = expp.tile([P, S], F32, tag="exp")
                            nc.scalar.activation(
                                exp_sb,
                                scores_sb,
                                Exp,
                                bias=negmx,
                                scale=1.0,
                                accum_out=sumv,
                            )
                            rec = smol.tile([P, 1], F32, tag="rec")
                            nc.vector.reciprocal(rec, sumv)
                            attn_sb = attnp.tile([P, S], F32, tag="attn")
                            nc.vector.tensor_scalar_mul(
                                attn_sb, in0=exp_sb, scalar1=rec
                            )
                            nc.sync.dma_start(
                                out=attn_o[h, ts(t, P), :], in_=attn_sb
                            )
                            # transpose attn -> attnT [s, lq]
                            attnt = attntp.tile([P, SC, P], F32, tag="attnt")
                            for g in range(0, SC, GS):
                                gn = min(GS, SC - g)
                                pt = tpsum2.tile([P, GS * P], F32, tag="pt2")
                                for jj in range(gn):
                                    nc.tensor.transpose(
                                        pt[:, ts(jj, P)],
                                        attn_sb[:, ts(g + jj, P)],
                                        ident,
                                    )
                                nc.scalar.copy(
                                    attnt[:, g : g + gn, :], pt[:, : gn * P]
                                )
                            attnts[h2] = attnt
                        # AV^T for the pair: out [dv, lq]; the two heads go
                        # to different PE column groups (tile_position) and
                        # psum partition halves, emitted adjacently to pack.
                        for j in range(SC):
                            for h2 in range(2):
                                po = 64 * h2
                                nc.tensor.matmul(
                                    ps_av[po : po + 64, :],
                                    lhsT=vn[:, j, 64 * (2 * hp + h2) : 64 * (2 * hp + h2) + 64],
                                    rhs=attnts[h2][:, j, :],
                                    start=(j == 0),
                                    stop=(j == SC - 1),
                                    tile_position=(0, po),
                                    skip_group_check=True,
                                )
                        nc.scalar.copy(avt[:, hp, :], ps_av)
                    # out projection for this lq tile
                    out_sb = outp.tile([P, D], F32, tag="out")
                    for g in range(D // ND):
                        ps_o = opsum.tile([P, ND], F32, tag="o")
                        for kk in range(KO):
                            nc.tensor.matmul(
                                ps_o,
                                lhsT=cast(avt[:, kk, :]),
                                rhs=cast(wo_sb[:, kk, ts(g, ND)]),
                                start=(kk == 0),
                                stop=(kk == KO - 1),
                            )
                        nc.scalar.copy(out_sb[:, ts(g, ND)], ps_o)
                    nc.sync.dma_start(out=out_o[ts(t, P), :], in_=out_sb)

    nc.compile()
    return nc


_NC_CACHE = {}


def _get_nc(**kw):
    key = tuple(sorted(kw.items()))
    if key not in _NC_CACHE:
        _NC_CACHE[key] = build_nc(**kw)
    return _NC_CACHE[key]


def make_in_maps(queries, keys, values, prev_logits, Wq, bq, Wk, bk, Wv, bv, Wo):
    scale = np.float32(1.0 / np.sqrt(DK))
    in_maps = []
    for c in range(N_CORES):
        b, g = c // 2, c % 2
        h0 = g * HPC
        in_maps.append(
            {
                "q": np.ascontiguousarray(queries[b]),
                "k": np.ascontiguousarray(keys[b]),
                "v": np.ascontiguousarray(values[b]),
                "prev": np.ascontiguousarray(prev_logits[b, h0 : h0 + HPC]),
                "wq": np.ascontiguousarray(Wq[:, h0 * DK : (h0 + HPC) * DK]) * scale,
                "wk": np.ascontiguousarray(Wk[:, h0 * DK : (h0 + HPC) * DK]),
                "wv": np.ascontiguousarray(Wv[:, h0 * DV : (h0 + HPC) * DV]),
                "wo": np.ascontiguousarray(Wo[h0 * DV : (h0 + HPC) * DV, :]),
                "bq": np.ascontiguousarray(bq[h0 * DK : (h0 + HPC) * DK]) * scale,
                "bk": np.ascontiguousarray(bk[h0 * DK : (h0 + HPC) * DK]),
                "bv": np.ascontiguousarray(bv[h0 * DV : (h0 + HPC) * DV]),
            }
        )
    return in_maps


def assemble(results, bo):
    """results: list of 8 dicts with scores_o/attn_o/out_o -> (out, attn, scores)."""
    scores = np.empty((B, H, L, S), np.float32)
    attn = np.empty((B, H, L, S), np.float32)
    out = np.empty((B, L, D), np.float32)
    for c in range(N_CORES):
        b, g = c // 2, c % 2
        h0 = g * HPC
        scores[b, h0 : h0 + HPC] = results[c]["scores_o"]
        attn[b, h0 : h0 + HPC] = results[c]["attn_o"]
    for b in range(B):
        out[b] = results[2 * b]["out_o"] + results[2 * b + 1]["out_o"] + bo
    return out, attn, scores


def run_on_device(inputs, trace=False, **spmd_kwargs):
    from concourse.bass_utils import run_bass_kernel_spmd

    nc = _get_nc()
    in_maps = make_in_maps(
        inputs["queries"],
        inputs["keys"],
        inputs["values"],
        inputs["prev_logits"],
        inputs["Wq"],
        inputs["bq"],
        inputs["Wk"],
        inputs["bk"],
        inputs["Wv"],
        inputs["bv"],
        inputs["Wo"],
    )
    res = run_bass_kernel_spmd(
        nc, in_maps, core_ids=list(range(N_CORES)), trace=trace, **spmd_kwargs
    )
    out, attn, scores = assemble(res.results, np.asarray(inputs["bo"]))
    return (out, attn, scores), res


def _host_mask_fixup(inputs, out, attn, scores):
    """Handle a non-trivial attn_mask (never hit for the spec'd inputs, where
    the mask is all-False) by recomputing attn/out on the host from the
    device-computed scores."""
    mask = np.asarray(inputs["attn_mask"])
    if not mask.any():
        return out, attn, scores
    masked = np.where(mask, np.float32(-1e9), scores)
    m = masked.max(axis=-1, keepdims=True)
    e = np.exp(masked - m)
    attn = (e / e.sum(axis=-1, keepdims=True)).astype(np.float32)
    values = np.asarray(inputs["values"])
    V = (values @ np.asarray(inputs["Wv"]) + np.asarray(inputs["bv"])).reshape(
        B, S, H, DV
    )
    av = np.einsum("bhls,bshd->blhd", attn, V).reshape(B, L, H * DV)
    out = (av @ np.asarray(inputs["Wo"]) + np.asarray(inputs["bo"])).astype(
        np.float32
    )
    return out, attn, scores


def kernel(**inputs):
    inputs = {k_: np.asarray(v_) for k_, v_ in inputs.items()}
    (out, attn, scores), _ = run_on_device(inputs)
    out, attn, scores = _host_mask_fixup(inputs, out, attn, scores)
    return out, attn, scores


# revision 15
# speedup vs baseline: 1.0103x; 1.0103x over previous
"""Trainium2 Bass kernel for AttentionLayerWithPrev.

Reference computation (B=4, L=S=1024, D=1024, H=16, DK=DV=64):
    Q = queries @ Wq + bq;  K = keys @ Wk + bk;  V = values @ Wv + bv
    scores = einsum(Q, K) / sqrt(DK) + prev_logits          [B, H, L, S]
    attn   = softmax(where(mask, -1e9, scores), axis=-1)    [B, H, L, S]
    out    = einsum(attn, V) @ Wo + bo                      [B, L, D]
Returns (out, attn, scores).

Sharding: 8 cores; core c handles batch b = c//2 and head group g = c%2
(heads 8g..8g+7).  Each core computes its slice of scores/attn fully and a
partial output projection (its 8 heads' contribution); the host sums the two
partials per batch and adds bo.
"""

import numpy as np

import concourse.bass as bass
import concourse.bacc as bacc
import concourse.tile as tile
from concourse import mybir
from concourse.bass import ts
from concourse.masks import make_identity

P = 128

# Full-problem constants (hardcoded per harness contract).
B, L, S = 4, 1024, 1024
D, H, DK, DV = 1024, 16, 64, 64
N_CORES = 8
HPC = H // (N_CORES // B)  # heads per core = 8

F32 = mybir.dt.float32


def build_nc(
    L=L,
    S=S,
    D=D,
    HPC=HPC,
    DK=DK,
    DV=DV,
    mm_dt=F32,
    enable_asserts=False,
):
    """Build the per-core Bass program.

    Per-core external tensors (heads pre-sliced / weights pre-scaled on host):
      q, k, v      [L|S, D]        inputs for this core's batch
      prev         [HPC, L, S]     prev_logits slice (this core's heads)
      wq, wk       [D, HPC*DK]     pre-scaled by 1/sqrt(DK) (wq only)
      wv           [D, HPC*DV]
      wo           [HPC*DV, D]
      bq, bk       [HPC*DK]        (bq pre-scaled by 1/sqrt(DK))
      bv           [HPC*DV]
      scores_o     [HPC, L, S]     output
      attn_o       [HPC, L, S]     output
      out_o        [L, D]          partial output projection (no bo)
    """
    HDK = HPC * DK
    HDV = HPC * DV
    assert D % P == 0 and L % P == 0 and S % P == 0
    assert HDK % P == 0 and HDV % P == 0 and DK == 64 and DV == 64
    DC = D // P      # contraction chunks for projections
    LT = L // P      # lq tiles
    SC = S // P      # s chunks (transpose/AV)
    MQ = HDK // P    # dk tiles of Qt/Kt (2 heads per tile)
    KO = HDV // P    # contraction chunks for out projection
    NS = min(512, S)   # scores matmul free chunk
    NL = min(512, L)   # projection free chunk
    ND = min(512, D)   # out projection free chunk
    GT = min(4, LT)  # transpose batch (input transposes)
    GS = min(4, SC)  # transpose batch (attn transposes)

    nc = bacc.Bacc(
        "TRN2",
        target_bir_lowering=False,
        debug=False,
        enable_asserts=enable_asserts,
        num_devices=1,
    )

    def mk_in(name, shape):
        return nc.dram_tensor(name, shape, F32, kind="ExternalInput").ap()

    def mk_out(name, shape):
        return nc.dram_tensor(name, shape, F32, kind="ExternalOutput").ap()

    q = mk_in("q", [L, D])
    k = mk_in("k", [S, D])
    v = mk_in("v", [S, D])
    prev = mk_in("prev", [HPC, L, S])
    wq = mk_in("wq", [D, HDK])
    wk = mk_in("wk", [D, HDK])
    wv = mk_in("wv", [D, HDV])
    wo = mk_in("wo", [HDV, D])
    bq = mk_in("bq", [HDK])
    bk = mk_in("bk", [HDK])
    bv = mk_in("bv", [HDV])
    scores_o = mk_out("scores_o", [HPC, L, S])
    attn_o = mk_out("attn_o", [HPC, L, S])
    out_o = mk_out("out_o", [L, D])

    Exp = mybir.ActivationFunctionType.Exp

    def cast(ap):
        return ap.bitcast(mm_dt) if mm_dt != F32 else ap

    with tile.TileContext(nc) as tc:
        with (
            tc.tile_pool(name="consts", bufs=1) as consts,
            tc.tile_pool(name="persist", bufs=1) as persist,
            tc.tile_pool(name="prevp", bufs=4) as prevp,
        ):
            ident = consts.tile([P, P], F32)
            make_identity(nc, ident)

            # Persistent SBUF tensors for the attention phase.
            qt = persist.tile([P, MQ, L], F32)     # Q^T  [dk, l]
            kt = persist.tile([P, MQ, S], F32)     # K^T  [dk, s]
            vn = persist.tile([P, SC, HDV], F32)   # V    [s, hdv]
            bq_sb = persist.tile([P, MQ], F32)
            bk_sb = persist.tile([P, MQ], F32)
            bv_bc = persist.tile([P, HDV], F32)    # bv broadcast over partitions

            nc.gpsimd.dma_start(
                out=bq_sb, in_=bq.rearrange("(m p) -> p m", p=P)
            )
            nc.gpsimd.dma_start(
                out=bk_sb, in_=bk.rearrange("(m p) -> p m", p=P)
            )
            bv_bcast_ap = bass.AP(
                tensor=bv.tensor, offset=bv.offset, ap=[[0, P], *bv.ap]
            )
            nc.gpsimd.dma_start(out=bv_bc, in_=bv_bcast_ap)

            # ---------------- Stage 1: transposes + QKV projections --------
            with (
                tc.tile_pool(name="wpool", bufs=2) as wpool,
                tc.tile_pool(name="xpool", bufs=2) as xpool,
                tc.tile_pool(name="xtpool", bufs=1) as xtpool,
                tc.tile_pool(name="tpsum", bufs=2, space="PSUM") as tpsum,
                tc.tile_pool(name="ppsum", bufs=2, space="PSUM") as ppsum,
            ):
                for xi, (src, wsrc) in enumerate([(q, wq), (k, wk), (v, wv)]):
                    xlt = (L if xi == 0 else S) // P
                    x_sb = xpool.tile([P, xlt, D], F32, tag="x")
                    src_t = src.rearrange("(i p) d -> p i d", p=P)
                    # per-tile DMAs so the first transposes start after 0.5MB
                    for i in range(xlt):
                        nc.sync.dma_start(out=x_sb[:, i, :], in_=src_t[:, i, :])
                    w_sb = wpool.tile([P, DC, HDK], F32, tag="w")
                    nc.sync.dma_start(
                        out=w_sb, in_=wsrc.rearrange("(c p) n -> p c n", p=P)
                    )
                    # Transpose x -> xT [d, l]
                    xT = xtpool.tile([P, DC, xlt * P], F32, tag="xt")
                    for j in range(DC):
                        for g in range(0, xlt, GT):
                            gn = min(GT, xlt - g)
                            pt = tpsum.tile([P, GT * P], F32, tag="pt")
                            for ii in range(gn):
                                nc.tensor.transpose(
                                    pt[:, ts(ii, P)],
                                    x_sb[:, g + ii, ts(j, P)],
                                    ident,
                                )
                            nc.scalar.copy(
                                xT[:, j, g * P : (g + gn) * P],
                                pt[:, : gn * P],
                            )
                    if xi < 2:
                        # Q^T / K^T = Wq^T @ x^T: out [dk, l]
                        dst = qt if xi == 0 else kt
                        bias = bq_sb if xi == 0 else bk_sb
                        nchunks = (xlt * P) // NL
                        for m in range(MQ):
                            for g in range(nchunks):
                                ps = ppsum.tile([P, NL], F32, tag="ps")
                                for kk in range(DC):
                                    nc.tensor.matmul(
                                        ps,
                                        lhsT=cast(w_sb[:, kk, ts(m, P)]),
                                        rhs=cast(xT[:, kk, ts(g, NL)]),
                                        start=(kk == 0),
                                        stop=(kk == DC - 1),
                                    )
                                nc.vector.tensor_scalar_add(
                                    dst[:, m, ts(g, NL)],
                                    in0=ps,
                                    scalar1=bias[:, m : m + 1],
                                )
                    else:
                        # V = v @ Wv: out [s, hdv]
                        for m in range(SC):
                            ps = ppsum.tile([P, HDV], F32, tag="psv")
                            for kk in range(DC):
                                nc.tensor.matmul(
                                    ps,
                                    lhsT=cast(xT[:, kk, ts(m, P)]),
                                    rhs=cast(w_sb[:, kk, :HDV]),
                                    start=(kk == 0),
                                    stop=(kk == DC - 1),
                                )
                            nc.vector.tensor_add(vn[:, m, :], ps, bv_bc)

            # ---------------- Stage 2: attention -------------------------
            with (
                tc.tile_pool(name="wop", bufs=1) as wop,
                tc.tile_pool(name="scorep", bufs=3) as scorep,
                tc.tile_pool(name="expp", bufs=2) as expp,
                tc.tile_pool(name="attnp", bufs=3) as attnp,
                tc.tile_pool(name="attntp", bufs=3) as attntp,
                tc.tile_pool(name="avtp", bufs=2) as avtp,
                tc.tile_pool(name="outp", bufs=2) as outp,
                tc.tile_pool(name="smol", bufs=12) as smol,
                tc.tile_pool(name="spsum", bufs=2, space="PSUM") as spsum,
                tc.tile_pool(name="tpsum2", bufs=2, space="PSUM") as tpsum2,
                tc.tile_pool(name="avpsum", bufs=1, space="PSUM") as avpsum,
                tc.tile_pool(name="opsum", bufs=1, space="PSUM") as opsum,
            ):
                wo_sb = wop.tile([P, KO, D], F32)  # Wo [hdv, d]
                nc.sync.dma_start(
                    out=wo_sb, in_=wo.rearrange("(c p) n -> p c n", p=P)
                )
                for t in range(LT):
                    avt = avtp.tile([P, KO, P], F32, tag="avt")
                    for hp in range(HPC // 2):
                        ps_av = avpsum.tile([P, P], F32, tag="av")
                        # scores matmuls for the head pair, emitted adjacently
                        # so the K=64 matmuls pack into different PE row
                        # groups (partition offsets 0/64) and run ~2x.
                        ps_pair = [
                            spsum.tile([P, S], F32, tag="s", name=f"ps_s{i}")
                            for i in range(2)
                        ]
                        for g in range(S // NS):
                            for h2 in range(2):
                                po = 64 * h2
                                nc.tensor.matmul(
                                    ps_pair[h2][:, ts(g, NS)],
                                    lhsT=cast(qt[po : po + 64, hp, ts(t, P)]),
                                    rhs=cast(kt[po : po + 64, hp, ts(g, NS)]),
                                    start=True,
                                    stop=True,
                                    tile_position=(po, 0),
                                )
                        attnts = [None, None]
                        for h2 in range(2):
                            h = 2 * hp + h2
                            po = 64 * h2
                            ps_s = ps_pair[h2]
                            prev_sb = prevp.tile([P, S], F32, tag="prev")
                            nc.sync.dma_start(
                                out=prev_sb, in_=prev[h, ts(t, P), :]
                            )
                            scores_sb = scorep.tile([P, S], F32, tag="sc")
                            nc.vector.tensor_add(scores_sb, ps_s, prev_sb)
                            nc.sync.dma_start(
                                out=scores_o[h, ts(t, P), :], in_=scores_sb
                            )
                            # softmax over free dim
                            mx = smol.tile([P, 1], F32, tag="mx")
                            nc.vector.reduce_max(
                                mx, scores_sb, axis=mybir.AxisListType.X
                            )
                            negmx = smol.tile([P, 1], F32, tag="negmx")
                            nc.vector.tensor_scalar_mul(negmx, in0=mx, scalar1=-1.0)
                            sumv = smol.tile([P, 1], F32, tag="sumv")
                            exp_sb = expp.tile([P, S], F32, tag="exp")
                            nc.scalar.activation(
                                exp_sb,
                                scores_sb,
                                Exp,
                                bias=negmx,
                                scale=1.0,
                                accum_out=sumv,
                            )
                            rec = smol.tile([P, 1], F32, tag="rec")
                            nc.vector.reciprocal(rec, sumv)
                            attn_sb = attnp.tile([P, S], F32, tag="attn")
                            nc.vector.tensor_scalar_mul(
                                attn_sb, in0=exp_sb, scalar1=rec
                            )
                            nc.sync.dma_start(
                                out=attn_o[h, ts(t, P), :], in_=attn_sb
                            )
                            # transpose attn -> attnT [s, lq]
                            attnt = attntp.tile([P, SC, P], F32, tag="attnt")
                            for g in range(0, SC, GS):
                                gn = min(GS, SC - g)
                                pt = tpsum2.tile([P, GS * P], F32, tag="pt2")
                                for jj in range(gn):
                                    nc.tensor.transpose(
                                        pt[:, ts(jj, P)],
                                        attn_sb[:, ts(g + jj, P)],
                                        ident,
                                    )
                                nc.scalar.copy(
                                    attnt[:, g : g + gn, :], pt[:, : gn * P]
                                )
                            attnts[h2] = attnt
                        # AV^T for the pair: out [dv, lq]; the two heads go
                        # to different PE column groups (tile_position) and
                        # psum partition halves, emitted adjacently to pack.
                        for j in range(SC):
                            for h2 in range(2):
                                po = 64 * h2
                                nc.tensor.matmul(
                                    ps_av[po : po + 64, :],
                                    lhsT=vn[:, j, 64 * (2 * hp + h2) : 64 * (2 * hp + h2) + 64],
                                    rhs=attnts[h2][:, j, :],
                                    start=(j == 0),
                                    stop=(j == SC - 1),
                                    tile_position=(0, po),
                                    skip_group_check=True,
                                )
                        nc.scalar.copy(avt[:, hp, :], ps_av)
                    # out projection for this lq tile
                    out_sb = outp.tile([P, D], F32, tag="out")
                    for g in range(D // ND):
                        ps_o = opsum.tile([P, ND], F32, tag="o")
                        for kk in range(KO):
                            nc.tensor.matmul(
                                ps_o,
                                lhsT=cast(avt[:, kk, :]),
                                rhs=cast(wo_sb[:, kk, ts(g, ND)]),
                                start=(kk == 0),
                                stop=(kk == KO - 1),
                            )
                        nc.scalar.copy(out_sb[:, ts(g, ND)], ps_o)
                    nc.sync.dma_start(out=out_o[ts(t, P), :], in_=out_sb)

    nc.compile()
    return nc


_NC_CACHE = {}


def _get_nc(**kw):
    key = tuple(sorted(kw.items()))
    if key not in _NC_CACHE:
        _NC_CACHE[key] = build_nc(**kw)
    return _NC_CACHE[key]


def make_in_maps(queries, keys, values, prev_logits, Wq, bq, Wk, bk, Wv, bv, Wo):
    scale = np.float32(1.0 / np.sqrt(DK))
    in_maps = []
    for c in range(N_CORES):
        b, g = c // 2, c % 2
        h0 = g * HPC
        in_maps.append(
            {
                "q": np.ascontiguousarray(queries[b]),
                "k": np.ascontiguousarray(keys[b]),
                "v": np.ascontiguousarray(values[b]),
                "prev": np.ascontiguousarray(prev_logits[b, h0 : h0 + HPC]),
                "wq": np.ascontiguousarray(Wq[:, h0 * DK : (h0 + HPC) * DK]) * scale,
                "wk": np.ascontiguousarray(Wk[:, h0 * DK : (h0 + HPC) * DK]),
                "wv": np.ascontiguousarray(Wv[:, h0 * DV : (h0 + HPC) * DV]),
                "wo": np.ascontiguousarray(Wo[h0 * DV : (h0 + HPC) * DV, :]),
                "bq": np.ascontiguousarray(bq[h0 * DK : (h0 + HPC) * DK]) * scale,
                "bk": np.ascontiguousarray(bk[h0 * DK : (h0 + HPC) * DK]),
                "bv": np.ascontiguousarray(bv[h0 * DV : (h0 + HPC) * DV]),
            }
        )
    return in_maps


def assemble(results, bo):
    """results: list of 8 dicts with scores_o/attn_o/out_o -> (out, attn, scores)."""
    scores = np.empty((B, H, L, S), np.float32)
    attn = np.empty((B, H, L, S), np.float32)
    out = np.empty((B, L, D), np.float32)
    for c in range(N_CORES):
        b, g = c // 2, c % 2
        h0 = g * HPC
        scores[b, h0 : h0 + HPC] = results[c]["scores_o"]
        attn[b, h0 : h0 + HPC] = results[c]["attn_o"]
    for b in range(B):
        out[b] = results[2 * b]["out_o"] + results[2 * b + 1]["out_o"] + bo
    return out, attn, scores


def run_on_device(inputs, trace=False, **spmd_kwargs):
    from concourse.bass_utils import run_bass_kernel_spmd

    nc = _get_nc()
    in_maps = make_in_maps(
        inputs["queries"],
        inputs["keys"],
        inputs["values"],
        inputs["prev_logits"],
        inputs["Wq"],
        inputs["bq"],
        inputs["Wk"],
        inputs["bk"],
        inputs["Wv"],
        inputs["bv"],
        inputs["Wo"],
    )
    res = run_bass_kernel_spmd(
        nc, in_maps, core_ids=list(range(N_CORES)), trace=trace, **spmd_kwargs
    )
    out, attn, scores = assemble(res.results, np.asarray(inputs["bo"]))
    return (out, attn, scores), res


def _host_mask_fixup(inputs, out, attn, scores):
    """Handle a non-trivial attn_mask (never hit for the spec'd inputs, where
    the mask is all-False) by recomputing attn/out on the host from the
    device-computed scores."""
    mask = np.asarray(inputs["attn_mask"])
    if not mask.any():
        return out, attn, scores
    masked = np.where(mask, np.float32(-1e9), scores)
    m = masked.max(axis=-1, keepdims=True)
    e = np.exp(masked - m)
    attn = (e / e.sum(axis=-1, keepdims=True)).astype(np.float32)
    values = np.asarray(inputs["values"])
    V = (values @ np.asarray(inputs["Wv"]) + np.asarray(inputs["bv"])).reshape(
        B, S, H, DV
    )
    av = np.einsum("bhls,bshd->blhd", attn, V).reshape(B, L, H * DV)
    out = (av @ np.asarray(inputs["Wo"]) + np.asarray(inputs["bo"])).astype(
        np.float32
    )
    return out, attn, scores


def kernel(**inputs):
    inputs = {k_: np.asarray(v_) for k_, v_ in inputs.items()}
    (out, attn, scores), _ = run_on_device(inputs)
    out, attn, scores = _host_mask_fixup(inputs, out, attn, scores)
    return out, attn, scores


# revision 16
# speedup vs baseline: 1.0495x; 1.0388x over previous
"""Trainium2 Bass kernel for AttentionLayerWithPrev.

Reference computation (B=4, L=S=1024, D=1024, H=16, DK=DV=64):
    Q = queries @ Wq + bq;  K = keys @ Wk + bk;  V = values @ Wv + bv
    scores = einsum(Q, K) / sqrt(DK) + prev_logits          [B, H, L, S]
    attn   = softmax(where(mask, -1e9, scores), axis=-1)    [B, H, L, S]
    out    = einsum(attn, V) @ Wo + bo                      [B, L, D]
Returns (out, attn, scores).

Sharding: 8 cores; core c handles batch b = c//2 and head group g = c%2
(heads 8g..8g+7).  Each core computes its slice of scores/attn fully and a
partial output projection (its 8 heads' contribution); the host sums the two
partials per batch and adds bo.
"""

import numpy as np

import concourse.bass as bass
import concourse.bacc as bacc
import concourse.tile as tile
from concourse import mybir
from concourse.bass import ts
from concourse.masks import make_identity

P = 128

# Full-problem constants (hardcoded per harness contract).
B, L, S = 4, 1024, 1024
D, H, DK, DV = 1024, 16, 64, 64
N_CORES = 8
HPC = H // (N_CORES // B)  # heads per core = 8

F32 = mybir.dt.float32


def build_nc(
    L=L,
    S=S,
    D=D,
    HPC=HPC,
    DK=DK,
    DV=DV,
    mm_dt=F32,
    enable_asserts=False,
):
    """Build the per-core Bass program.

    Per-core external tensors (heads pre-sliced / weights pre-scaled on host):
      q, k, v      [L|S, D]        inputs for this core's batch
      prev         [HPC, L, S]     prev_logits slice (this core's heads)
      wq, wk       [D, HPC*DK]     pre-scaled by 1/sqrt(DK) (wq only)
      wv           [D, HPC*DV]
      wo           [HPC*DV, D]
      bq, bk       [HPC*DK]        (bq pre-scaled by 1/sqrt(DK))
      bv           [HPC*DV]
      scores_o     [HPC, L, S]     output
      attn_o       [HPC, L, S]     output
      out_o        [L, D]          partial output projection (no bo)
    """
    HDK = HPC * DK
    HDV = HPC * DV
    assert D % P == 0 and L % P == 0 and S % P == 0
    assert HDK % P == 0 and HDV % P == 0 and DK == 64 and DV == 64
    DC = D // P      # contraction chunks for projections
    LT = L // P      # lq tiles
    SC = S // P      # s chunks (transpose/AV)
    MQ = HDK // P    # dk tiles of Qt/Kt (2 heads per tile)
    KO = HDV // P    # contraction chunks for out projection
    NS = min(512, S)   # scores matmul free chunk
    NL = min(512, L)   # projection free chunk
    ND = min(512, D)   # out projection free chunk
    GT = min(4, LT)  # transpose batch (input transposes)
    GS = min(4, SC)  # transpose batch (attn transposes)

    nc = bacc.Bacc(
        "TRN2",
        target_bir_lowering=False,
        debug=False,
        enable_asserts=enable_asserts,
        num_devices=1,
    )

    def mk_in(name, shape):
        return nc.dram_tensor(name, shape, F32, kind="ExternalInput").ap()

    def mk_out(name, shape):
        return nc.dram_tensor(name, shape, F32, kind="ExternalOutput").ap()

    q = mk_in("q", [L, D])
    k = mk_in("k", [S, D])
    v = mk_in("v", [S, D])
    prev = mk_in("prev", [HPC, L, S])
    wq = mk_in("wq", [D, HDK])
    wk = mk_in("wk", [D, HDK])
    wv = mk_in("wv", [D, HDV])
    wo = mk_in("wo", [HDV, D])
    bq = mk_in("bq", [HDK])
    bk = mk_in("bk", [HDK])
    bv = mk_in("bv", [HDV])
    scores_o = mk_out("scores_o", [HPC, L, S])
    attn_o = mk_out("attn_o", [HPC, L, S])
    out_o = mk_out("out_o", [L, D])

    Exp = mybir.ActivationFunctionType.Exp

    def cast(ap):
        return ap.bitcast(mm_dt) if mm_dt != F32 else ap

    with tile.TileContext(nc) as tc:
        with (
            tc.tile_pool(name="consts", bufs=1) as consts,
            tc.tile_pool(name="persist", bufs=1) as persist,
            tc.tile_pool(name="prevp", bufs=4) as prevp,
        ):
            ident = consts.tile([P, P], F32)
            make_identity(nc, ident)

            # Persistent SBUF tensors for the attention phase.
            qt = persist.tile([P, MQ, L], F32)     # Q^T  [dk, l]
            kt = persist.tile([P, MQ, S], F32)     # K^T  [dk, s]
            vn = persist.tile([P, SC, HDV], F32)   # V    [s, hdv]
            bq_sb = persist.tile([P, MQ], F32)
            bk_sb = persist.tile([P, MQ], F32)
            bv_bc = persist.tile([P, HDV], F32)    # bv broadcast over partitions

            nc.gpsimd.dma_start(
                out=bq_sb, in_=bq.rearrange("(m p) -> p m", p=P)
            )
            nc.gpsimd.dma_start(
                out=bk_sb, in_=bk.rearrange("(m p) -> p m", p=P)
            )
            bv_bcast_ap = bass.AP(
                tensor=bv.tensor, offset=bv.offset, ap=[[0, P], *bv.ap]
            )
            nc.gpsimd.dma_start(out=bv_bc, in_=bv_bcast_ap)

            # ---------------- Stage 1: transposes + QKV projections --------
            with (
                tc.tile_pool(name="wpool", bufs=2) as wpool,
                tc.tile_pool(name="xpool", bufs=2) as xpool,
                tc.tile_pool(name="xtpool", bufs=1) as xtpool,
                tc.tile_pool(name="tpsum", bufs=2, space="PSUM") as tpsum,
                tc.tile_pool(name="ppsum", bufs=2, space="PSUM") as ppsum,
            ):
                for xi, (src, wsrc) in enumerate([(q, wq), (k, wk), (v, wv)]):
                    xlt = (L if xi == 0 else S) // P
                    x_sb = xpool.tile([P, xlt, D], F32, tag="x")
                    src_t = src.rearrange("(i p) d -> p i d", p=P)
                    # per-tile DMAs so the first transposes start after 0.5MB
                    for i in range(xlt):
                        nc.sync.dma_start(out=x_sb[:, i, :], in_=src_t[:, i, :])
                    w_sb = wpool.tile([P, DC, HDK], F32, tag="w")
                    nc.sync.dma_start(
                        out=w_sb, in_=wsrc.rearrange("(c p) n -> p c n", p=P)
                    )
                    # Transpose x -> xT [d, l]
                    xT = xtpool.tile([P, DC, xlt * P], F32, tag="xt")
                    for j in range(DC):
                        for g in range(0, xlt, GT):
                            gn = min(GT, xlt - g)
                            pt = tpsum.tile([P, GT * P], F32, tag="pt")
                            for ii in range(gn):
                                nc.tensor.transpose(
                                    pt[:, ts(ii, P)],
                                    x_sb[:, g + ii, ts(j, P)],
                                    ident,
                                )
                            nc.scalar.copy(
                                xT[:, j, g * P : (g + gn) * P],
                                pt[:, : gn * P],
                            )
                    if xi < 2:
                        # Q^T / K^T = Wq^T @ x^T: out [dk, l]
                        dst = qt if xi == 0 else kt
                        bias = bq_sb if xi == 0 else bk_sb
                        nchunks = (xlt * P) // NL
                        for m in range(MQ):
                            for g in range(nchunks):
                                ps = ppsum.tile([P, NL], F32, tag="ps")
                                for kk in range(DC):
                                    nc.tensor.matmul(
                                        ps,
                                        lhsT=cast(w_sb[:, kk, ts(m, P)]),
                                        rhs=cast(xT[:, kk, ts(g, NL)]),
                                        start=(kk == 0),
                                        stop=(kk == DC - 1),
                                    )
                                nc.vector.tensor_scalar_add(
                                    dst[:, m, ts(g, NL)],
                                    in0=ps,
                                    scalar1=bias[:, m : m + 1],
                                )
                    else:
                        # V = v @ Wv: out [s, hdv]
                        for m in range(SC):
                            ps = ppsum.tile([P, HDV], F32, tag="psv")
                            for kk in range(DC):
                                nc.tensor.matmul(
                                    ps,
                                    lhsT=cast(xT[:, kk, ts(m, P)]),
                                    rhs=cast(w_sb[:, kk, :HDV]),
                                    start=(kk == 0),
                                    stop=(kk == DC - 1),
                                )
                            nc.vector.tensor_add(vn[:, m, :], ps, bv_bc)

            # ---------------- Stage 2: attention -------------------------
            with (
                tc.tile_pool(name="wop", bufs=1) as wop,
                tc.tile_pool(name="scorep", bufs=3) as scorep,
                tc.tile_pool(name="expp", bufs=3) as expp,
                tc.tile_pool(name="attnp", bufs=6) as attnp,
                tc.tile_pool(name="attntp", bufs=4) as attntp,
                tc.tile_pool(name="avtp", bufs=2) as avtp,
                tc.tile_pool(name="outp", bufs=2) as outp,
                tc.tile_pool(name="smol", bufs=12) as smol,
                tc.tile_pool(name="spsum", bufs=5, space="PSUM") as spsum,
                tc.tile_pool(name="tpsum2", bufs=2, space="PSUM") as tpsum2,
                tc.tile_pool(name="avop", bufs=1, space="PSUM") as avop,
            ):
                wo_sb = wop.tile([P, KO, D], F32)  # Wo [hdv, d]
                nc.sync.dma_start(
                    out=wo_sb, in_=wo.rearrange("(c p) n -> p c n", p=P)
                )

                def scores_softmax(t, hp, avt):
                    """Scores matmuls + softmax chain for head pair hp of lq
                    tile t. Returns the two attn tiles for the deferred
                    transpose+AV stage."""
                    # one [P, NS] psum bank per (h2, g); freed by per-half adds
                    ps_pair = [
                        [
                            spsum.tile([P, NS], F32, tag="s", name=f"ps_s{i}{g}")
                            for g in range(S // NS)
                        ]
                        for i in range(2)
                    ]
                    for g in range(S // NS):
                        for h2 in range(2):
                            po = 64 * h2
                            nc.tensor.matmul(
                                ps_pair[h2][g],
                                lhsT=cast(qt[po : po + 64, hp, ts(t, P)]),
                                rhs=cast(kt[po : po + 64, hp, ts(g, NS)]),
                                start=True,
                                stop=True,
                                tile_position=(po, 0),
                            )
                    attns = [None, None]
                    for h2 in range(2):
                        h = 2 * hp + h2
                        prev_sb = prevp.tile([P, S], F32, tag="prev")
                        nc.sync.dma_start(out=prev_sb, in_=prev[h, ts(t, P), :])
                        scores_sb = scorep.tile([P, S], F32, tag="sc")
                        for g in range(S // NS):
                            nc.vector.tensor_add(
                                scores_sb[:, ts(g, NS)],
                                ps_pair[h2][g],
                                prev_sb[:, ts(g, NS)],
                            )
                        nc.sync.dma_start(
                            out=scores_o[h, ts(t, P), :], in_=scores_sb
                        )
                        # softmax over free dim
                        mx = smol.tile([P, 1], F32, tag="mx")
                        nc.vector.reduce_max(
                            mx, scores_sb, axis=mybir.AxisListType.X
                        )
                        negmx = smol.tile([P, 1], F32, tag="negmx")
                        nc.vector.tensor_scalar_mul(negmx, in0=mx, scalar1=-1.0)
                        sumv = smol.tile([P, 1], F32, tag="sumv")
                        exp_sb = expp.tile([P, S], F32, tag="exp")
                        nc.scalar.activation(
                            exp_sb,
                            scores_sb,
                            Exp,
                            bias=negmx,
                            scale=1.0,
                            accum_out=sumv,
                        )
                        rec = smol.tile([P, 1], F32, tag="rec")
                        nc.vector.reciprocal(rec, sumv)
                        attn_sb = attnp.tile([P, S], F32, tag="attn")
                        nc.vector.tensor_scalar_mul(attn_sb, in0=exp_sb, scalar1=rec)
                        nc.sync.dma_start(
                            out=attn_o[h, ts(t, P), :], in_=attn_sb
                        )
                        attns[h2] = attn_sb
                    return attns

                def transpose_av(hp, attns, avt):
                    """PE transposes of attn + AV matmuls for head pair hp."""
                    attnts = [None, None]
                    for h2 in range(2):
                        attnt = attntp.tile([P, SC, P], F32, tag="attnt")
                        for g in range(0, SC, GS):
                            gn = min(GS, SC - g)
                            pt = tpsum2.tile([P, GS * P], F32, tag="pt2")
                            for jj in range(gn):
                                nc.tensor.transpose(
                                    pt[:, ts(jj, P)],
                                    attns[h2][:, ts(g + jj, P)],
                                    ident,
                                )
                            nc.scalar.copy(
                                attnt[:, g : g + gn, :], pt[:, : gn * P]
                            )
                        attnts[h2] = attnt
                    # AV^T for the pair: the two heads go to different PE
                    # column groups and psum partition halves, adjacent so
                    # they pack.
                    ps_av = avop.tile([P, ND], F32, tag="avo", name="ps_av")
                    for j in range(SC):
                        for h2 in range(2):
                            po = 64 * h2
                            nc.tensor.matmul(
                                ps_av[po : po + 64, :P],
                                lhsT=vn[:, j, 64 * (2 * hp + h2) : 64 * (2 * hp + h2) + 64],
                                rhs=attnts[h2][:, j, :],
                                start=(j == 0),
                                stop=(j == SC - 1),
                                tile_position=(0, po),
                                skip_group_check=True,
                            )
                    nc.scalar.copy(avt[:, hp, :], ps_av[:, :P])

                for t in range(LT):
                    avt = avtp.tile([P, KO, P], F32, tag="avt")
                    # software pipeline: transposes+AV of pair hp run while
                    # pair hp+1's scores matmuls fill the PE, hiding the
                    # softmax-chain latency.
                    pend = None
                    for hp in range(HPC // 2):
                        attns = scores_softmax(t, hp, avt)
                        if pend is not None:
                            transpose_av(pend[0], pend[1], avt)
                        pend = (hp, attns)
                    transpose_av(pend[0], pend[1], avt)
                    # out projection for this lq tile
                    out_sb = outp.tile([P, D], F32, tag="out")
                    for g in range(D // ND):
                        ps_o = avop.tile([P, ND], F32, tag="avo", name="ps_o")
                        for kk in range(KO):
                            nc.tensor.matmul(
                                ps_o,
                                lhsT=cast(avt[:, kk, :]),
                                rhs=cast(wo_sb[:, kk, ts(g, ND)]),
                                start=(kk == 0),
                                stop=(kk == KO - 1),
                            )
                        nc.scalar.copy(out_sb[:, ts(g, ND)], ps_o)
                    nc.sync.dma_start(out=out_o[ts(t, P), :], in_=out_sb)

    nc.compile()
    return nc


_NC_CACHE = {}


def _get_nc(**kw):
    key = tuple(sorted(kw.items()))
    if key not in _NC_CACHE:
        _NC_CACHE[key] = build_nc(**kw)
    return _NC_CACHE[key]


def make_in_maps(queries, keys, values, prev_logits, Wq, bq, Wk, bk, Wv, bv, Wo):
    scale = np.float32(1.0 / np.sqrt(DK))
    in_maps = []
    for c in range(N_CORES):
        b, g = c // 2, c % 2
        h0 = g * HPC
        in_maps.append(
            {
                "q": np.ascontiguousarray(queries[b]),
                "k": np.ascontiguousarray(keys[b]),
                "v": np.ascontiguousarray(values[b]),
                "prev": np.ascontiguousarray(prev_logits[b, h0 : h0 + HPC]),
                "wq": np.ascontiguousarray(Wq[:, h0 * DK : (h0 + HPC) * DK]) * scale,
                "wk": np.ascontiguousarray(Wk[:, h0 * DK : (h0 + HPC) * DK]),
                "wv": np.ascontiguousarray(Wv[:, h0 * DV : (h0 + HPC) * DV]),
                "wo": np.ascontiguousarray(Wo[h0 * DV : (h0 + HPC) * DV, :]),
                "bq": np.ascontiguousarray(bq[h0 * DK : (h0 + HPC) * DK]) * scale,
                "bk": np.ascontiguousarray(bk[h0 * DK : (h0 + HPC) * DK]),
                "bv": np.ascontiguousarray(bv[h0 * DV : (h0 + HPC) * DV]),
            }
        )
    return in_maps


def assemble(results, bo):
    """results: list of 8 dicts with scores_o/attn_o/out_o -> (out, attn, scores)."""
    scores = np.empty((B, H, L, S), np.float32)
    attn = np.empty((B, H, L, S), np.float32)
    out = np.empty((B, L, D), np.float32)
    for c in range(N_CORES):
        b, g = c // 2, c % 2
        h0 = g * HPC
        scores[b, h0 : h0 + HPC] = results[c]["scores_o"]
        attn[b, h0 : h0 + HPC] = results[c]["attn_o"]
    for b in range(B):
        out[b] = results[2 * b]["out_o"] + results[2 * b + 1]["out_o"] + bo
    return out, attn, scores


def run_on_device(inputs, trace=False, **spmd_kwargs):
    from concourse.bass_utils import run_bass_kernel_spmd

    nc = _get_nc()
    in_maps = make_in_maps(
        inputs["queries"],
        inputs["keys"],
        inputs["values"],
        inputs["prev_logits"],
        inputs["Wq"],
        inputs["bq"],
        inputs["Wk"],
        inputs["bk"],
        inputs["Wv"],
        inputs["bv"],
        inputs["Wo"],
    )
    res = run_bass_kernel_spmd(
        nc, in_maps, core_ids=list(range(N_CORES)), trace=trace, **spmd_kwargs
    )
    out, attn, scores = assemble(res.results, np.asarray(inputs["bo"]))
    return (out, attn, scores), res


def _host_mask_fixup(inputs, out, attn, scores):
    """Handle a non-trivial attn_mask (never hit for the spec'd inputs, where
    the mask is all-False) by recomputing attn/out on the host from the
    device-computed scores."""
    mask = np.asarray(inputs["attn_mask"])
    if not mask.any():
        return out, attn, scores
    masked = np.where(mask, np.float32(-1e9), scores)
    m = masked.max(axis=-1, keepdims=True)
    e = np.exp(masked - m)
    attn = (e / e.sum(axis=-1, keepdims=True)).astype(np.float32)
    values = np.asarray(inputs["values"])
    V = (values @ np.asarray(inputs["Wv"]) + np.asarray(inputs["bv"])).reshape(
        B, S, H, DV
    )
    av = np.einsum("bhls,bshd->blhd", attn, V).reshape(B, L, H * DV)
    out = (av @ np.asarray(inputs["Wo"]) + np.asarray(inputs["bo"])).astype(
        np.float32
    )
    return out, attn, scores


def kernel(**inputs):
    inputs = {k_: np.asarray(v_) for k_, v_ in inputs.items()}
    (out, attn, scores), _ = run_on_device(inputs)
    out, attn, scores = _host_mask_fixup(inputs, out, attn, scores)
    return out, attn, scores


# revision 18
# speedup vs baseline: 1.0614x; 1.0114x over previous
"""Trainium2 Bass kernel for AttentionLayerWithPrev.

Reference computation (B=4, L=S=1024, D=1024, H=16, DK=DV=64):
    Q = queries @ Wq + bq;  K = keys @ Wk + bk;  V = values @ Wv + bv
    scores = einsum(Q, K) / sqrt(DK) + prev_logits          [B, H, L, S]
    attn   = softmax(where(mask, -1e9, scores), axis=-1)    [B, H, L, S]
    out    = einsum(attn, V) @ Wo + bo                      [B, L, D]
Returns (out, attn, scores).

Sharding: 8 cores; core c handles batch b = c//2 and head group g = c%2
(heads 8g..8g+7).  Each core computes its slice of scores/attn fully and a
partial output projection (its 8 heads' contribution); the host sums the two
partials per batch and adds bo.
"""

import numpy as np

import concourse.bass as bass
import concourse.bacc as bacc
import concourse.tile as tile
from concourse import mybir
from concourse.bass import ts
from concourse.masks import make_identity

P = 128

# Full-problem constants (hardcoded per harness contract).
B, L, S = 4, 1024, 1024
D, H, DK, DV = 1024, 16, 64, 64
N_CORES = 8
HPC = H // (N_CORES // B)  # heads per core = 8

F32 = mybir.dt.float32


def build_nc(
    L=L,
    S=S,
    D=D,
    HPC=HPC,
    DK=DK,
    DV=DV,
    mm_dt=F32,
    enable_asserts=False,
):
    """Build the per-core Bass program.

    Per-core external tensors (heads pre-sliced / weights pre-scaled on host):
      q, k, v      [L|S, D]        inputs for this core's batch
      prev         [HPC, L, S]     prev_logits slice (this core's heads)
      wq, wk       [D, HPC*DK]     pre-scaled by 1/sqrt(DK) (wq only)
      wv           [D, HPC*DV]
      wo           [HPC*DV, D]
      bq, bk       [HPC*DK]        (bq pre-scaled by 1/sqrt(DK))
      bv           [HPC*DV]
      scores_o     [HPC, L, S]     output
      attn_o       [HPC, L, S]     output
      out_o        [L, D]          partial output projection (no bo)
    """
    HDK = HPC * DK
    HDV = HPC * DV
    assert D % P == 0 and L % P == 0 and S % P == 0
    assert HDK % P == 0 and HDV % P == 0 and DK == 64 and DV == 64
    DC = D // P      # contraction chunks for projections
    LT = L // P      # lq tiles
    SC = S // P      # s chunks (transpose/AV)
    MQ = HDK // P    # dk tiles of Qt/Kt (2 heads per tile)
    KO = HDV // P    # contraction chunks for out projection
    NS = min(512, S)   # scores matmul free chunk
    NL = min(512, L)   # projection free chunk
    ND = min(512, D)   # out projection free chunk
    GT = min(4, LT)  # transpose batch (input transposes)
    GS = min(4, SC)  # transpose batch (attn transposes)

    nc = bacc.Bacc(
        "TRN2",
        target_bir_lowering=False,
        debug=False,
        enable_asserts=enable_asserts,
        num_devices=1,
    )

    def mk_in(name, shape):
        return nc.dram_tensor(name, shape, F32, kind="ExternalInput").ap()

    def mk_out(name, shape):
        return nc.dram_tensor(name, shape, F32, kind="ExternalOutput").ap()

    q = mk_in("q", [L, D])
    k = mk_in("k", [S, D])
    v = mk_in("v", [S, D])
    prev = mk_in("prev", [HPC, L, S])
    wq = mk_in("wq", [D, HDK])
    wk = mk_in("wk", [D, HDK])
    wv = mk_in("wv", [D, HDV])
    wo = mk_in("wo", [HDV, D])
    bq = mk_in("bq", [HDK])
    bk = mk_in("bk", [HDK])
    bv = mk_in("bv", [HDV])
    scores_o = mk_out("scores_o", [HPC, L, S])
    attn_o = mk_out("attn_o", [HPC, L, S])
    out_o = mk_out("out_o", [L, D])

    Exp = mybir.ActivationFunctionType.Exp

    def cast(ap):
        return ap.bitcast(mm_dt) if mm_dt != F32 else ap

    with tile.TileContext(nc) as tc:
        with (
            tc.tile_pool(name="consts", bufs=1) as consts,
            tc.tile_pool(name="persist", bufs=1) as persist,
            tc.tile_pool(name="prevp", bufs=4) as prevp,
        ):
            ident = consts.tile([P, P], F32)
            make_identity(nc, ident)

            # Persistent SBUF tensors for the attention phase.
            qt = persist.tile([P, MQ, L], F32)     # Q^T  [dk, l]
            kt = persist.tile([P, MQ, S], F32)     # K^T  [dk, s]
            vn = persist.tile([P, SC, HDV], F32)   # V    [s, hdv]
            bq_sb = persist.tile([P, MQ], F32)
            bk_sb = persist.tile([P, MQ], F32)
            bv_bc = persist.tile([P, HDV], F32)    # bv broadcast over partitions

            nc.gpsimd.dma_start(
                out=bq_sb, in_=bq.rearrange("(m p) -> p m", p=P)
            )
            nc.gpsimd.dma_start(
                out=bk_sb, in_=bk.rearrange("(m p) -> p m", p=P)
            )
            bv_bcast_ap = bass.AP(
                tensor=bv.tensor, offset=bv.offset, ap=[[0, P], *bv.ap]
            )
            nc.gpsimd.dma_start(out=bv_bc, in_=bv_bcast_ap)

            # ---------------- Stage 1: transposes + QKV projections --------
            with (
                tc.tile_pool(name="wpool", bufs=2) as wpool,
                tc.tile_pool(name="xpool", bufs=2) as xpool,
                tc.tile_pool(name="xtpool", bufs=1) as xtpool,
                tc.tile_pool(name="tpsum", bufs=2, space="PSUM") as tpsum,
                tc.tile_pool(name="ppsum", bufs=2, space="PSUM") as ppsum,
            ):
                for xi, (src, wsrc) in enumerate([(q, wq), (k, wk), (v, wv)]):
                    xlt = (L if xi == 0 else S) // P
                    x_sb = xpool.tile([P, xlt, D], F32, tag="x")
                    src_t = src.rearrange("(i p) d -> p i d", p=P)
                    # per-tile DMAs so the first transposes start after 0.5MB
                    for i in range(xlt):
                        nc.sync.dma_start(out=x_sb[:, i, :], in_=src_t[:, i, :])
                    w_sb = wpool.tile([P, DC, HDK], F32, tag="w")
                    nc.sync.dma_start(
                        out=w_sb, in_=wsrc.rearrange("(c p) n -> p c n", p=P)
                    )
                    # Transpose x -> xT [d, l]
                    xT = xtpool.tile([P, DC, xlt * P], F32, tag="xt")
                    for j in range(DC):
                        for g in range(0, xlt, GT):
                            gn = min(GT, xlt - g)
                            pt = tpsum.tile([P, GT * P], F32, tag="pt")
                            for ii in range(gn):
                                nc.tensor.transpose(
                                    pt[:, ts(ii, P)],
                                    x_sb[:, g + ii, ts(j, P)],
                                    ident,
                                )
                            nc.scalar.copy(
                                xT[:, j, g * P : (g + gn) * P],
                                pt[:, : gn * P],
                            )
                    if xi < 2:
                        # Q^T / K^T = Wq^T @ x^T: out [dk, l]
                        dst = qt if xi == 0 else kt
                        bias = bq_sb if xi == 0 else bk_sb
                        nchunks = (xlt * P) // NL
                        for m in range(MQ):
                            for g in range(nchunks):
                                ps = ppsum.tile([P, NL], F32, tag="ps")
                                for kk in range(DC):
                                    nc.tensor.matmul(
                                        ps,
                                        lhsT=cast(w_sb[:, kk, ts(m, P)]),
                                        rhs=cast(xT[:, kk, ts(g, NL)]),
                                        start=(kk == 0),
                                        stop=(kk == DC - 1),
                                    )
                                nc.vector.tensor_scalar_add(
                                    dst[:, m, ts(g, NL)],
                                    in0=ps,
                                    scalar1=bias[:, m : m + 1],
                                )
                    else:
                        # V = v @ Wv: out [s, hdv]
                        for m in range(SC):
                            ps = ppsum.tile([P, HDV], F32, tag="psv")
                            for kk in range(DC):
                                nc.tensor.matmul(
                                    ps,
                                    lhsT=cast(xT[:, kk, ts(m, P)]),
                                    rhs=cast(w_sb[:, kk, :HDV]),
                                    start=(kk == 0),
                                    stop=(kk == DC - 1),
                                )
                            nc.vector.tensor_add(vn[:, m, :], ps, bv_bc)

            # ---------------- Stage 2: attention -------------------------
            with (
                tc.tile_pool(name="wop", bufs=1) as wop,
                tc.tile_pool(name="scorep", bufs=3) as scorep,
                tc.tile_pool(name="expp", bufs=3) as expp,
                tc.tile_pool(name="attnp", bufs=6) as attnp,
                tc.tile_pool(name="attntp", bufs=4) as attntp,
                tc.tile_pool(name="avtp", bufs=3) as avtp,
                tc.tile_pool(name="outp", bufs=2) as outp,
                tc.tile_pool(name="smol", bufs=12) as smol,
                tc.tile_pool(name="spsum", bufs=5, space="PSUM") as spsum,
                tc.tile_pool(name="tpsum2", bufs=2, space="PSUM") as tpsum2,
                tc.tile_pool(name="avop", bufs=1, space="PSUM") as avop,
            ):
                wo_sb = wop.tile([P, KO, D], F32)  # Wo [hdv, d]
                nc.sync.dma_start(
                    out=wo_sb, in_=wo.rearrange("(c p) n -> p c n", p=P)
                )

                def scores_softmax(t, hp, avt):
                    """Scores matmuls + softmax chain for head pair hp of lq
                    tile t. Returns the two attn tiles for the deferred
                    transpose+AV stage."""
                    # one [P, NS] psum bank per (h2, g); freed by per-half adds
                    ps_pair = [
                        [
                            spsum.tile([P, NS], F32, tag="s", name=f"ps_s{i}{g}")
                            for g in range(S // NS)
                        ]
                        for i in range(2)
                    ]
                    for g in range(S // NS):
                        for h2 in range(2):
                            po = 64 * h2
                            nc.tensor.matmul(
                                ps_pair[h2][g],
                                lhsT=cast(qt[po : po + 64, hp, ts(t, P)]),
                                rhs=cast(kt[po : po + 64, hp, ts(g, NS)]),
                                start=True,
                                stop=True,
                                tile_position=(po, 0),
                            )
                    attns = [None, None]
                    for h2 in range(2):
                        h = 2 * hp + h2
                        prev_sb = prevp.tile([P, S], F32, tag="prev")
                        nc.sync.dma_start(out=prev_sb, in_=prev[h, ts(t, P), :])
                        scores_sb = scorep.tile([P, S], F32, tag="sc")
                        for g in range(S // NS):
                            nc.vector.tensor_add(
                                scores_sb[:, ts(g, NS)],
                                ps_pair[h2][g],
                                prev_sb[:, ts(g, NS)],
                            )
                        nc.sync.dma_start(
                            out=scores_o[h, ts(t, P), :], in_=scores_sb
                        )
                        # softmax over free dim
                        mx = smol.tile([P, 1], F32, tag="mx")
                        nc.vector.reduce_max(
                            mx, scores_sb, axis=mybir.AxisListType.X
                        )
                        negmx = smol.tile([P, 1], F32, tag="negmx")
                        nc.vector.tensor_scalar_mul(negmx, in0=mx, scalar1=-1.0)
                        sumv = smol.tile([P, 1], F32, tag="sumv")
                        exp_sb = expp.tile([P, S], F32, tag="exp")
                        nc.scalar.activation(
                            exp_sb,
                            scores_sb,
                            Exp,
                            bias=negmx,
                            scale=1.0,
                            accum_out=sumv,
                        )
                        rec = smol.tile([P, 1], F32, tag="rec")
                        nc.vector.reciprocal(rec, sumv)
                        attn_sb = attnp.tile([P, S], F32, tag="attn")
                        nc.vector.tensor_scalar_mul(attn_sb, in0=exp_sb, scalar1=rec)
                        nc.sync.dma_start(
                            out=attn_o[h, ts(t, P), :], in_=attn_sb
                        )
                        attns[h2] = attn_sb
                    return attns

                def transpose_av(hp, attns, avt):
                    """PE transposes of attn + AV matmuls for head pair hp."""
                    attnts = [None, None]
                    for h2 in range(2):
                        attnt = attntp.tile([P, SC, P], F32, tag="attnt")
                        for g in range(0, SC, GS):
                            gn = min(GS, SC - g)
                            pt = tpsum2.tile([P, GS * P], F32, tag="pt2")
                            for jj in range(gn):
                                nc.tensor.transpose(
                                    pt[:, ts(jj, P)],
                                    attns[h2][:, ts(g + jj, P)],
                                    ident,
                                )
                            nc.scalar.copy(
                                attnt[:, g : g + gn, :], pt[:, : gn * P]
                            )
                        attnts[h2] = attnt
                    # AV^T for the pair: the two heads go to different PE
                    # column groups and psum partition halves, adjacent so
                    # they pack.
                    ps_av = avop.tile([P, ND], F32, tag="avo", name="ps_av")
                    for j in range(SC):
                        for h2 in range(2):
                            po = 64 * h2
                            nc.tensor.matmul(
                                ps_av[po : po + 64, :P],
                                lhsT=vn[:, j, 64 * (2 * hp + h2) : 64 * (2 * hp + h2) + 64],
                                rhs=attnts[h2][:, j, :],
                                start=(j == 0),
                                stop=(j == SC - 1),
                                tile_position=(0, po),
                                skip_group_check=True,
                            )
                    nc.scalar.copy(avt[:, hp, :], ps_av[:, :P])

                for t in range(LT):
                    avt = avtp.tile([P, KO, P], F32, tag="avt")
                    # software pipeline: transposes+AV of pair hp run while
                    # pair hp+1's scores matmuls fill the PE, hiding the
                    # softmax-chain latency.
                    pend = None
                    for hp in range(HPC // 2):
                        attns = scores_softmax(t, hp, avt)
                        if pend is not None:
                            transpose_av(pend[0], pend[1], avt)
                        pend = (hp, attns)
                    transpose_av(pend[0], pend[1], avt)
                    # out projection for this lq tile
                    out_sb = outp.tile([P, D], F32, tag="out")
                    for g in range(D // ND):
                        ps_o = tpsum2.tile([P, GS * P], F32, tag="pt2", name="ps_o")[:, :ND]
                        for kk in range(KO):
                            nc.tensor.matmul(
                                ps_o,
                                lhsT=cast(avt[:, kk, :]),
                                rhs=cast(wo_sb[:, kk, ts(g, ND)]),
                                start=(kk == 0),
                                stop=(kk == KO - 1),
                            )
                        nc.scalar.copy(out_sb[:, ts(g, ND)], ps_o)
                    nc.sync.dma_start(out=out_o[ts(t, P), :], in_=out_sb)

    nc.compile()
    return nc


_NC_CACHE = {}


def _get_nc(**kw):
    key = tuple(sorted(kw.items()))
    if key not in _NC_CACHE:
        _NC_CACHE[key] = build_nc(**kw)
    return _NC_CACHE[key]


def make_in_maps(queries, keys, values, prev_logits, Wq, bq, Wk, bk, Wv, bv, Wo):
    scale = np.float32(1.0 / np.sqrt(DK))
    in_maps = []
    for c in range(N_CORES):
        b, g = c // 2, c % 2
        h0 = g * HPC
        in_maps.append(
            {
                "q": np.ascontiguousarray(queries[b]),
                "k": np.ascontiguousarray(keys[b]),
                "v": np.ascontiguousarray(values[b]),
                "prev": np.ascontiguousarray(prev_logits[b, h0 : h0 + HPC]),
                "wq": np.ascontiguousarray(Wq[:, h0 * DK : (h0 + HPC) * DK]) * scale,
                "wk": np.ascontiguousarray(Wk[:, h0 * DK : (h0 + HPC) * DK]),
                "wv": np.ascontiguousarray(Wv[:, h0 * DV : (h0 + HPC) * DV]),
                "wo": np.ascontiguousarray(Wo[h0 * DV : (h0 + HPC) * DV, :]),
                "bq": np.ascontiguousarray(bq[h0 * DK : (h0 + HPC) * DK]) * scale,
                "bk": np.ascontiguousarray(bk[h0 * DK : (h0 + HPC) * DK]),
                "bv": np.ascontiguousarray(bv[h0 * DV : (h0 + HPC) * DV]),
            }
        )
    return in_maps


def assemble(results, bo):
    """results: list of 8 dicts with scores_o/attn_o/out_o -> (out, attn, scores)."""
    scores = np.empty((B, H, L, S), np.float32)
    attn = np.empty((B, H, L, S), np.float32)
    out = np.empty((B, L, D), np.float32)
    for c in range(N_CORES):
        b, g = c // 2, c % 2
        h0 = g * HPC
        scores[b, h0 : h0 + HPC] = results[c]["scores_o"]
        attn[b, h0 : h0 + HPC] = results[c]["attn_o"]
    for b in range(B):
        out[b] = results[2 * b]["out_o"] + results[2 * b + 1]["out_o"] + bo
    return out, attn, scores


def run_on_device(inputs, trace=False, **spmd_kwargs):
    from concourse.bass_utils import run_bass_kernel_spmd

    nc = _get_nc()
    in_maps = make_in_maps(
        inputs["queries"],
        inputs["keys"],
        inputs["values"],
        inputs["prev_logits"],
        inputs["Wq"],
        inputs["bq"],
        inputs["Wk"],
        inputs["bk"],
        inputs["Wv"],
        inputs["bv"],
        inputs["Wo"],
    )
    res = run_bass_kernel_spmd(
        nc, in_maps, core_ids=list(range(N_CORES)), trace=trace, **spmd_kwargs
    )
    out, attn, scores = assemble(res.results, np.asarray(inputs["bo"]))
    return (out, attn, scores), res


def _host_mask_fixup(inputs, out, attn, scores):
    """Handle a non-trivial attn_mask (never hit for the spec'd inputs, where
    the mask is all-False) by recomputing attn/out on the host from the
    device-computed scores."""
    mask = np.asarray(inputs["attn_mask"])
    if not mask.any():
        return out, attn, scores
    masked = np.where(mask, np.float32(-1e9), scores)
    m = masked.max(axis=-1, keepdims=True)
    e = np.exp(masked - m)
    attn = (e / e.sum(axis=-1, keepdims=True)).astype(np.float32)
    values = np.asarray(inputs["values"])
    V = (values @ np.asarray(inputs["Wv"]) + np.asarray(inputs["bv"])).reshape(
        B, S, H, DV
    )
    av = np.einsum("bhls,bshd->blhd", attn, V).reshape(B, L, H * DV)
    out = (av @ np.asarray(inputs["Wo"]) + np.asarray(inputs["bo"])).astype(
        np.float32
    )
    return out, attn, scores


def kernel(**inputs):
    inputs = {k_: np.asarray(v_) for k_, v_ in inputs.items()}
    (out, attn, scores), _ = run_on_device(inputs)
    out, attn, scores = _host_mask_fixup(inputs, out, attn, scores)
    return out, attn, scores


# revision 19
# speedup vs baseline: 1.0694x; 1.0075x over previous
"""Trainium2 Bass kernel for AttentionLayerWithPrev.

Reference computation (B=4, L=S=1024, D=1024, H=16, DK=DV=64):
    Q = queries @ Wq + bq;  K = keys @ Wk + bk;  V = values @ Wv + bv
    scores = einsum(Q, K) / sqrt(DK) + prev_logits          [B, H, L, S]
    attn   = softmax(where(mask, -1e9, scores), axis=-1)    [B, H, L, S]
    out    = einsum(attn, V) @ Wo + bo                      [B, L, D]
Returns (out, attn, scores).

Sharding: 8 cores; core c handles batch b = c//2 and head group g = c%2
(heads 8g..8g+7).  Each core computes its slice of scores/attn fully and a
partial output projection (its 8 heads' contribution); the host sums the two
partials per batch and adds bo.
"""

import numpy as np

import concourse.bass as bass
import concourse.bacc as bacc
import concourse.tile as tile
from concourse import mybir
from concourse.bass import ts
from concourse.masks import make_identity

P = 128

# Full-problem constants (hardcoded per harness contract).
B, L, S = 4, 1024, 1024
D, H, DK, DV = 1024, 16, 64, 64
N_CORES = 8
HPC = H // (N_CORES // B)  # heads per core = 8

F32 = mybir.dt.float32


def build_nc(
    L=L,
    S=S,
    D=D,
    HPC=HPC,
    DK=DK,
    DV=DV,
    mm_dt=F32,
    enable_asserts=False,
):
    """Build the per-core Bass program.

    Per-core external tensors (heads pre-sliced / weights pre-scaled on host):
      q, k, v      [L|S, D]        inputs for this core's batch
      prev         [HPC, L, S]     prev_logits slice (this core's heads)
      wq, wk       [D, HPC*DK]     pre-scaled by 1/sqrt(DK) (wq only)
      wv           [D, HPC*DV]
      wo           [HPC*DV, D]
      bq, bk       [HPC*DK]        (bq pre-scaled by 1/sqrt(DK))
      bv           [HPC*DV]
      scores_o     [HPC, L, S]     output
      attn_o       [HPC, L, S]     output
      out_o        [L, D]          partial output projection (no bo)
    """
    HDK = HPC * DK
    HDV = HPC * DV
    assert D % P == 0 and L % P == 0 and S % P == 0
    assert HDK % P == 0 and HDV % P == 0 and DK == 64 and DV == 64
    DC = D // P      # contraction chunks for projections
    LT = L // P      # lq tiles
    SC = S // P      # s chunks (transpose/AV)
    MQ = HDK // P    # dk tiles of Qt/Kt (2 heads per tile)
    KO = HDV // P    # contraction chunks for out projection
    NS = min(512, S)   # scores matmul free chunk
    NL = min(512, L)   # projection free chunk
    ND = min(512, D)   # out projection free chunk
    GT = min(4, LT)  # transpose batch (input transposes)
    GS = min(4, SC)  # transpose batch (attn transposes)

    nc = bacc.Bacc(
        "TRN2",
        target_bir_lowering=False,
        debug=False,
        enable_asserts=enable_asserts,
        num_devices=1,
    )

    def mk_in(name, shape):
        return nc.dram_tensor(name, shape, F32, kind="ExternalInput").ap()

    def mk_out(name, shape):
        return nc.dram_tensor(name, shape, F32, kind="ExternalOutput").ap()

    q = mk_in("q", [L, D])
    k = mk_in("k", [S, D])
    v = mk_in("v", [S, D])
    prev = mk_in("prev", [HPC, L, S])
    wq = mk_in("wq", [D, HDK])
    wk = mk_in("wk", [D, HDK])
    wv = mk_in("wv", [D, HDV])
    wo = mk_in("wo", [HDV, D])
    bq = mk_in("bq", [HDK])
    bk = mk_in("bk", [HDK])
    bv = mk_in("bv", [HDV])
    scores_o = mk_out("scores_o", [HPC, L, S])
    attn_o = mk_out("attn_o", [HPC, L, S])
    out_o = mk_out("out_o", [L, D])

    Exp = mybir.ActivationFunctionType.Exp

    def cast(ap):
        return ap.bitcast(mm_dt) if mm_dt != F32 else ap

    with tile.TileContext(nc) as tc:
        with (
            tc.tile_pool(name="consts", bufs=1) as consts,
            tc.tile_pool(name="persist", bufs=1) as persist,
            tc.tile_pool(name="prevp", bufs=4) as prevp,
        ):
            ident = consts.tile([P, P], F32)
            make_identity(nc, ident)

            # Persistent SBUF tensors for the attention phase.
            qt = persist.tile([P, MQ, L], F32)     # Q^T  [dk, l]
            kt = persist.tile([P, MQ, S], F32)     # K^T  [dk, s]
            vn = persist.tile([P, SC, HDV], F32)   # V    [s, hdv]
            bq_sb = persist.tile([P, MQ], F32)
            bk_sb = persist.tile([P, MQ], F32)
            bv_bc = persist.tile([P, HDV], F32)    # bv broadcast over partitions

            nc.gpsimd.dma_start(
                out=bq_sb, in_=bq.rearrange("(m p) -> p m", p=P)
            )
            nc.gpsimd.dma_start(
                out=bk_sb, in_=bk.rearrange("(m p) -> p m", p=P)
            )
            bv_bcast_ap = bass.AP(
                tensor=bv.tensor, offset=bv.offset, ap=[[0, P], *bv.ap]
            )
            nc.gpsimd.dma_start(out=bv_bc, in_=bv_bcast_ap)

            # ---------------- Stage 1: transposes + QKV projections --------
            with (
                tc.tile_pool(name="wpool", bufs=2) as wpool,
                tc.tile_pool(name="xpool", bufs=2) as xpool,
                tc.tile_pool(name="xtpool", bufs=1) as xtpool,
                tc.tile_pool(name="tpsum", bufs=2, space="PSUM") as tpsum,
                tc.tile_pool(name="ppsum", bufs=2, space="PSUM") as ppsum,
            ):
                for xi, (src, wsrc) in enumerate([(q, wq), (k, wk), (v, wv)]):
                    xlt = (L if xi == 0 else S) // P
                    x_sb = xpool.tile([P, xlt, D], F32, tag="x")
                    src_t = src.rearrange("(i p) d -> p i d", p=P)
                    # per-tile DMAs so the first transposes start after 0.5MB
                    for i in range(xlt):
                        nc.sync.dma_start(out=x_sb[:, i, :], in_=src_t[:, i, :])
                    w_sb = wpool.tile([P, DC, HDK], F32, tag="w")
                    nc.sync.dma_start(
                        out=w_sb, in_=wsrc.rearrange("(c p) n -> p c n", p=P)
                    )
                    # Transpose x -> xT [d, l]
                    xT = xtpool.tile([P, DC, xlt * P], F32, tag="xt")
                    for j in range(DC):
                        for g in range(0, xlt, GT):
                            gn = min(GT, xlt - g)
                            pt = tpsum.tile([P, GT * P], F32, tag="pt")
                            for ii in range(gn):
                                nc.tensor.transpose(
                                    pt[:, ts(ii, P)],
                                    x_sb[:, g + ii, ts(j, P)],
                                    ident,
                                )
                            nc.scalar.copy(
                                xT[:, j, g * P : (g + gn) * P],
                                pt[:, : gn * P],
                            )
                    if xi < 2:
                        # Q^T / K^T = Wq^T @ x^T: out [dk, l]
                        dst = qt if xi == 0 else kt
                        bias = bq_sb if xi == 0 else bk_sb
                        nchunks = (xlt * P) // NL
                        for m in range(MQ):
                            for g in range(nchunks):
                                ps = ppsum.tile([P, NL], F32, tag="ps")
                                for kk in range(DC):
                                    nc.tensor.matmul(
                                        ps,
                                        lhsT=cast(w_sb[:, kk, ts(m, P)]),
                                        rhs=cast(xT[:, kk, ts(g, NL)]),
                                        start=(kk == 0),
                                        stop=(kk == DC - 1),
                                    )
                                nc.vector.tensor_scalar_add(
                                    dst[:, m, ts(g, NL)],
                                    in0=ps,
                                    scalar1=bias[:, m : m + 1],
                                )
                    else:
                        # V = v @ Wv: out [s, hdv]
                        for m in range(SC):
                            ps = ppsum.tile([P, HDV], F32, tag="psv")
                            for kk in range(DC):
                                nc.tensor.matmul(
                                    ps,
                                    lhsT=cast(xT[:, kk, ts(m, P)]),
                                    rhs=cast(w_sb[:, kk, :HDV]),
                                    start=(kk == 0),
                                    stop=(kk == DC - 1),
                                )
                            nc.vector.tensor_add(vn[:, m, :], ps, bv_bc)

            # ---------------- Stage 2: attention -------------------------
            with (
                tc.tile_pool(name="wop", bufs=1) as wop,
                tc.tile_pool(name="scorep", bufs=3) as scorep,
                tc.tile_pool(name="expp", bufs=3) as expp,
                tc.tile_pool(name="attnp", bufs=6) as attnp,
                tc.tile_pool(name="attntp", bufs=4) as attntp,
                tc.tile_pool(name="avtp", bufs=3) as avtp,
                tc.tile_pool(name="outp", bufs=2) as outp,
                tc.tile_pool(name="smol", bufs=12) as smol,
                tc.tile_pool(name="spsum", bufs=5, space="PSUM") as spsum,
                tc.tile_pool(name="tpsum2", bufs=2, space="PSUM") as tpsum2,
                tc.tile_pool(name="avop", bufs=1, space="PSUM") as avop,
            ):
                wo_sb = wop.tile([P, KO, D], F32)  # Wo [hdv, d]
                nc.sync.dma_start(
                    out=wo_sb, in_=wo.rearrange("(c p) n -> p c n", p=P)
                )

                def scores_softmax(t, hp, avt):
                    """Scores matmuls + softmax chain for head pair hp of lq
                    tile t. Returns the two attn tiles for the deferred
                    transpose+AV stage."""
                    # one [P, NS] psum bank per (h2, g); freed by per-half adds
                    ps_pair = [
                        [
                            spsum.tile([P, NS], F32, tag="s", name=f"ps_s{i}{g}")
                            for g in range(S // NS)
                        ]
                        for i in range(2)
                    ]
                    for g in range(S // NS):
                        for h2 in range(2):
                            po = 64 * h2
                            nc.tensor.matmul(
                                ps_pair[h2][g],
                                lhsT=cast(qt[po : po + 64, hp, ts(t, P)]),
                                rhs=cast(kt[po : po + 64, hp, ts(g, NS)]),
                                start=True,
                                stop=True,
                                tile_position=(po, 0),
                            )
                    attns = [None, None]
                    for h2 in range(2):
                        h = 2 * hp + h2
                        prev_sb = prevp.tile([P, S], F32, tag="prev")
                        nc.sync.dma_start(out=prev_sb, in_=prev[h, ts(t, P), :])
                        scores_sb = scorep.tile([P, S], F32, tag="sc")
                        for g in range(S // NS):
                            nc.vector.tensor_add(
                                scores_sb[:, ts(g, NS)],
                                ps_pair[h2][g],
                                prev_sb[:, ts(g, NS)],
                            )
                        nc.sync.dma_start(
                            out=scores_o[h, ts(t, P), :], in_=scores_sb
                        )
                        # softmax over free dim
                        mx = smol.tile([P, 1], F32, tag="mx")
                        nc.vector.reduce_max(
                            mx, scores_sb, axis=mybir.AxisListType.X
                        )
                        negmx = smol.tile([P, 1], F32, tag="negmx")
                        nc.vector.tensor_scalar_mul(negmx, in0=mx, scalar1=-1.0)
                        sumv = smol.tile([P, 1], F32, tag="sumv")
                        exp_sb = expp.tile([P, S], F32, tag="exp")
                        nc.scalar.activation(
                            exp_sb,
                            scores_sb,
                            Exp,
                            bias=negmx,
                            scale=1.0,
                            accum_out=sumv,
                        )
                        rec = smol.tile([P, 1], F32, tag="rec")
                        nc.vector.reciprocal(rec, sumv)
                        attn_sb = attnp.tile([P, S], F32, tag="attn")
                        nc.vector.tensor_scalar_mul(attn_sb, in0=exp_sb, scalar1=rec)
                        nc.sync.dma_start(
                            out=attn_o[h, ts(t, P), :], in_=attn_sb
                        )
                        attns[h2] = attn_sb
                    return attns

                def transpose_av(hp, attns, avt):
                    """PE transposes of attn + AV matmuls for head pair hp."""
                    attnts = [None, None]
                    for h2 in range(2):
                        attnt = attntp.tile([P, SC, P], F32, tag="attnt")
                        for g in range(0, SC, GS):
                            gn = min(GS, SC - g)
                            pt = tpsum2.tile([P, GS * P], F32, tag="pt2")
                            for jj in range(gn):
                                nc.tensor.transpose(
                                    pt[:, ts(jj, P)],
                                    attns[h2][:, ts(g + jj, P)],
                                    ident,
                                )
                            nc.scalar.copy(
                                attnt[:, g : g + gn, :], pt[:, : gn * P]
                            )
                        attnts[h2] = attnt
                    # AV^T for the pair: the two heads go to different PE
                    # column groups and psum partition halves, adjacent so
                    # they pack.
                    ps_av = avop.tile([P, ND], F32, tag="avo", name="ps_av")
                    for j in range(SC):
                        for h2 in range(2):
                            po = 64 * h2
                            nc.tensor.matmul(
                                ps_av[po : po + 64, :P],
                                lhsT=vn[:, j, 64 * (2 * hp + h2) : 64 * (2 * hp + h2) + 64],
                                rhs=attnts[h2][:, j, :],
                                start=(j == 0),
                                stop=(j == SC - 1),
                                tile_position=(0, po),
                                skip_group_check=True,
                            )
                    nc.scalar.copy(avt[:, hp, :], ps_av[:, :P])

                def outproj(t, avt):
                    out_sb = outp.tile([P, D], F32, tag="out")
                    for g in range(D // ND):
                        ps_o = tpsum2.tile([P, GS * P], F32, tag="pt2", name="ps_o")[:, :ND]
                        for kk in range(KO):
                            nc.tensor.matmul(
                                ps_o,
                                lhsT=cast(avt[:, kk, :]),
                                rhs=cast(wo_sb[:, kk, ts(g, ND)]),
                                start=(kk == 0),
                                stop=(kk == KO - 1),
                            )
                        nc.scalar.copy(out_sb[:, ts(g, ND)], ps_o)
                    nc.sync.dma_start(out=out_o[ts(t, P), :], in_=out_sb)

                # software pipeline across the whole (t, hp) sequence: the
                # transposes+AV of pair N (and the out projection at each
                # tile boundary) are emitted after pair N+1's scores
                # matmuls, so the softmax-chain latency and the tile
                # epilogue hide behind dense PE scores work.
                last_hp = HPC // 2 - 1
                pend = None
                for t in range(LT):
                    avt = avtp.tile([P, KO, P], F32, tag="avt")
                    for hp in range(HPC // 2):
                        attns = scores_softmax(t, hp, avt)
                        if pend is not None:
                            p_t, p_hp, p_attns, p_avt = pend
                            transpose_av(p_hp, p_attns, p_avt)
                            if p_hp == last_hp:
                                outproj(p_t, p_avt)
                        pend = (t, hp, attns, avt)
                p_t, p_hp, p_attns, p_avt = pend
                transpose_av(p_hp, p_attns, p_avt)
                outproj(p_t, p_avt)

    nc.compile()
    return nc


_NC_CACHE = {}


def _get_nc(**kw):
    key = tuple(sorted(kw.items()))
    if key not in _NC_CACHE:
        _NC_CACHE[key] = build_nc(**kw)
    return _NC_CACHE[key]


def make_in_maps(queries, keys, values, prev_logits, Wq, bq, Wk, bk, Wv, bv, Wo):
    scale = np.float32(1.0 / np.sqrt(DK))
    in_maps = []
    for c in range(N_CORES):
        b, g = c // 2, c % 2
        h0 = g * HPC
        in_maps.append(
            {
                "q": np.ascontiguousarray(queries[b]),
                "k": np.ascontiguousarray(keys[b]),
                "v": np.ascontiguousarray(values[b]),
                "prev": np.ascontiguousarray(prev_logits[b, h0 : h0 + HPC]),
                "wq": np.ascontiguousarray(Wq[:, h0 * DK : (h0 + HPC) * DK]) * scale,
                "wk": np.ascontiguousarray(Wk[:, h0 * DK : (h0 + HPC) * DK]),
                "wv": np.ascontiguousarray(Wv[:, h0 * DV : (h0 + HPC) * DV]),
                "wo": np.ascontiguousarray(Wo[h0 * DV : (h0 + HPC) * DV, :]),
                "bq": np.ascontiguousarray(bq[h0 * DK : (h0 + HPC) * DK]) * scale,
                "bk": np.ascontiguousarray(bk[h0 * DK : (h0 + HPC) * DK]),
                "bv": np.ascontiguousarray(bv[h0 * DV : (h0 + HPC) * DV]),
            }
        )
    return in_maps


def assemble(results, bo):
    """results: list of 8 dicts with scores_o/attn_o/out_o -> (out, attn, scores)."""
    scores = np.empty((B, H, L, S), np.float32)
    attn = np.empty((B, H, L, S), np.float32)
    out = np.empty((B, L, D), np.float32)
    for c in range(N_CORES):
        b, g = c // 2, c % 2
        h0 = g * HPC
        scores[b, h0 : h0 + HPC] = results[c]["scores_o"]
        attn[b, h0 : h0 + HPC] = results[c]["attn_o"]
    for b in range(B):
        out[b] = results[2 * b]["out_o"] + results[2 * b + 1]["out_o"] + bo
    return out, attn, scores


def run_on_device(inputs, trace=False, **spmd_kwargs):
    from concourse.bass_utils import run_bass_kernel_spmd

    nc = _get_nc()
    in_maps = make_in_maps(
        inputs["queries"],
        inputs["keys"],
        inputs["values"],
        inputs["prev_logits"],
        inputs["Wq"],
        inputs["bq"],
        inputs["Wk"],
        inputs["bk"],
        inputs["Wv"],
        inputs["bv"],
        inputs["Wo"],
    )
    res = run_bass_kernel_spmd(
        nc, in_maps, core_ids=list(range(N_CORES)), trace=trace, **spmd_kwargs
    )
    out, attn, scores = assemble(res.results, np.asarray(inputs["bo"]))
    return (out, attn, scores), res


def _host_mask_fixup(inputs, out, attn, scores):
    """Handle a non-trivial attn_mask (never hit for the spec'd inputs, where
    the mask is all-False) by recomputing attn/out on the host from the
    device-computed scores."""
    mask = np.asarray(inputs["attn_mask"])
    if not mask.any():
        return out, attn, scores
    masked = np.where(mask, np.float32(-1e9), scores)
    m = masked.max(axis=-1, keepdims=True)
    e = np.exp(masked - m)
    attn = (e / e.sum(axis=-1, keepdims=True)).astype(np.float32)
    values = np.asarray(inputs["values"])
    V = (values @ np.asarray(inputs["Wv"]) + np.asarray(inputs["bv"])).reshape(
        B, S, H, DV
    )
    av = np.einsum("bhls,bshd->blhd", attn, V).reshape(B, L, H * DV)
    out = (av @ np.asarray(inputs["Wo"]) + np.asarray(inputs["bo"])).astype(
        np.float32
    )
    return out, attn, scores


def kernel(**inputs):
    inputs = {k_: np.asarray(v_) for k_, v_ in inputs.items()}
    (out, attn, scores), _ = run_on_device(inputs)
    out, attn, scores = _host_mask_fixup(inputs, out, attn, scores)
    return out, attn, scores
